# revision 1
# baseline (speedup 1.0000x reference)
"""Multi-head attention (QKV proj + RoPE + causal SDPA + out proj) on 8 TRN2 cores.

Sharding: core c = 4*b + g handles batch b (of 2) and head-group g (of 4, i.e.
4 heads = 512 feature dims). Per-core device kernel computes, for its batch:
    q/k/v projections for its 4 heads (column-sharded w_q / w_kv),
    RoPE on q and k, causal softmax attention,
    partial output projection with its 512 rows of w_o (+ bias on g==0 cores).
Host gathers by summing the 4 per-group partials per batch (the "all-reduce"
for the row-sharded w_o, done as the unshard step).

Device dataflow is feature-major: the host passes x pre-transposed (xT[e, s],
a pure layout change); projections produce qT/kT in [head_dim, seq] layout and
v in [seq, head_dim] layout, which is exactly what the S^T = K·Q^T and
O^T = V^T·P^T matmuls need — no transposes anywhere on device.

Causality is exploited at block granularity: for the diagonal key block the
S/exp/PV/rowsum work is restricted to the valid q-range, and only the leading
128 q-columns (the triangle) need a mask multiply.
"""

import os
import sys

import numpy as np

sys.path.insert(0, "/opt/trn_rl_repo")

EMB = 2048
SEQ = 2048
N_HEAD = 16
HD = 128
BATCH = 2
N_CORES = 8
GROUPS = 4  # head groups (tensor-parallel dimension)
HPG = N_HEAD // GROUPS  # heads per group = 4
DPG = HPG * HD  # feature dims per group = 512
NE = EMB // 128  # 16 e-blocks
SCALE = float(HD) ** -0.5


def _host_tables(seq):
    """cos / sign-folded sin RoPE tables in [d, s] layout + triangle mask."""
    d = HD
    inv = 1.0 / (10000.0 ** (np.arange(0, d, 2, dtype=np.float64) / d))  # [64]
    pos = np.arange(seq, dtype=np.float64)[None, :] * inv[:, None]  # [64, s]
    ang = np.concatenate([pos, pos], axis=0)  # [128, s]
    cos_t = np.cos(ang).astype(np.float32)
    sin = np.sin(ang)
    # rot is built as a plain partition swap (rot[0:64]=q[64:128], rot[64:128]=q[0:64]);
    # the rotate_half sign lives in the sin table instead.
    sinm = np.concatenate([-sin[:64], sin[64:]], axis=0).astype(np.float32)
    # triangle mask for the diagonal 128x128 block: keep (ko, qo) iff qo >= ko
    ko = np.arange(128)[:, None]
    qo = np.arange(128)[None, :]
    mask_t = (qo >= ko).astype(np.float32)
    return cos_t, sinm, mask_t


def build(seq=SEQ, has_bias=False):
    """Build the per-core Bass program. Returns the compiled Bacc module."""
    import concourse.bacc as bacc
    import concourse.tile as tile
    from concourse import mybir

    f32 = mybir.dt.float32

    assert seq % 512 == 0
    nj = seq // 512  # 512-wide q chunks

    nc = bacc.Bacc("TRN2", target_bir_lowering=False, debug=False,
                   num_devices=N_CORES, name="mha8")

    xt_d = nc.dram_tensor("xt", [EMB, seq], f32, kind="ExternalInput")
    wq_d = nc.dram_tensor("wq", [EMB, DPG], f32, kind="ExternalInput")
    wk_d = nc.dram_tensor("wk", [EMB, DPG], f32, kind="ExternalInput")
    wv_d = nc.dram_tensor("wv", [EMB, DPG], f32, kind="ExternalInput")
    wo_d = nc.dram_tensor("wo", [DPG, EMB], f32, kind="ExternalInput")
    bo_d = nc.dram_tensor("bo", [1, EMB], f32, kind="ExternalInput")
    cos_d = nc.dram_tensor("cosT", [HD, seq], f32, kind="ExternalInput")
    sinm_d = nc.dram_tensor("sinM", [HD, seq], f32, kind="ExternalInput")
    mask_d = nc.dram_tensor("maskT", [128, 128], f32, kind="ExternalInput")
    out_d = nc.dram_tensor("out", [seq, EMB], f32, kind="ExternalOutput")

    with tile.TileContext(nc) as tc:
        _emit(nc, tc, tile, mybir, seq, nj,
              xt_d, wq_d, wk_d, wv_d, wo_d, bo_d, cos_d, sinm_d, mask_d, out_d,
              has_bias)
    nc.compile()
    return nc


def _emit(nc, tc, tile, mybir, seq, nj,
          xt_d, wq_d, wk_d, wv_d, wo_d, bo_d, cos_d, sinm_d, mask_d, out_d,
          has_bias):
    from contextlib import ExitStack

    f32 = mybir.dt.float32
    bf16 = mybir.dt.bfloat16
    EXP = mybir.ActivationFunctionType.Exp
    nsb = seq // 128

    ctx = ExitStack()
    with ctx:
        persist = ctx.enter_context(tc.tile_pool(name="persist", bufs=1))
        stage_p = ctx.enter_context(tc.tile_pool(name="stage", bufs=2))

        # right-side, phase-2 scoped: tables + projection weights
        ph2 = ExitStack()
        ph2_pool = ph2.enter_context(tc.tile_pool(name="ph2", bufs=1, side="right"))
        xt_pool = ph2.enter_context(tc.tile_pool(name="xt", bufs=2, side="right"))

        # ---- constants ----
        ones_col = persist.tile([128, 1], bf16, name="ones_col")
        nc.vector.memset(ones_col, 1.0)
        mask_sb = persist.tile([128, 128], bf16, name="mask_sb")
        cos_sb = ph2_pool.tile([128, seq], bf16, name="cos_sb")
        sinm_sb = ph2_pool.tile([128, seq], bf16, name="sinm_sb")

        def load_tables():
            # emitted after round-0's weight/x loads: rope/mask consumers run
            # well after the first projection chains, so keep these DMAs out
            # of the critical head of the queues
            mst = stage_p.tile([128, 128], f32, name="mst", tag="stg_w")
            nc.scalar.dma_start(mst, mask_d[:])
            nc.vector.tensor_copy(mask_sb, mst)
            for tbl_d, tbl_sb, nm in ((cos_d, cos_sb, "c"), (sinm_d, sinm_sb, "s")):
                for half in range(2):
                    hs = slice(half * (seq // 2), (half + 1) * (seq // 2))
                    st = stage_p.tile([128, seq // 2], f32, name=f"tstg{nm}{half}",
                                      tag="stg_x")
                    eng = nc.scalar if half else nc.sync
                    eng.dma_start(st, tbl_d[:, hs])
                    nc.vector.tensor_copy(tbl_sb[:, hs], st)

        # persistent activations
        kt = persist.tile([128, HPG, seq], bf16, name="kt")    # [d, h, s]
        yt = persist.tile([128, HPG, seq], bf16, name="yt")
        v_sb = persist.tile([128, nsb, DPG], bf16, name="v_sb")  # [s_in, blk, d]

        # w_q/w_k/w_v -> bf16 [128, NE, DPG], interleaved with round-0 x loads
        w_sb = {}
        for nm in ("wq", "wk", "wv"):
            w_sb[nm] = ph2_pool.tile([128, NE, DPG], bf16, name=f"{nm}_sb")

        def _copy(eng, out, in_):
            if eng is nc.scalar:
                nc.scalar.copy(out, in_)
            else:
                eng.tensor_copy(out, in_)

        def load_w(nm, wd, e2, dma_eng, cp_eng):
            # two e-tiles per DMA: DRAM rows [256*e2, 256*e2+256) -> [128, 2, 512]
            st = stage_p.tile([128, 2, DPG], f32, name=f"wst_{nm}{e2}",
                              tag="stg_w3", bufs=3)
            src = wd[e2 * 256:(e2 + 1) * 256, :].rearrange("(a p) d -> p a d", p=128)
            dma_eng.dma_start(st, src)
            _copy(cp_eng, w_sb[nm][:, 2 * e2:2 * e2 + 2, :], st)

        def load_xt(xt_j, j, e2):
            # two e-tiles per DMA: [256, 512] slab -> [128, 2, 512]
            st = stage_p.tile([128, 2, 512], f32, name=f"xst_{j}_{e2}", tag="stg_xs")
            eng = nc.sync if e2 % 2 == 0 else nc.scalar
            src = xt_d[e2 * 256:(e2 + 1) * 256,
                       j * 512:(j + 1) * 512].rearrange("(a p) s -> p a s", p=128)
            eng.dma_start(st, src)
            nc.vector.tensor_copy(xt_j[:, 2 * e2:2 * e2 + 2, :], st)

        qtj_pool = ctx.enter_context(tc.tile_pool(name="qtj", bufs=2))
        rope_pool = ctx.enter_context(tc.tile_pool(name="rope", bufs=2))
        pt_pool = ctx.enter_context(tc.tile_pool(name="pt", bufs=4))
        sm_pool = ctx.enter_context(tc.tile_pool(name="sm", bufs=2))
        out_pool = ctx.enter_context(tc.tile_pool(name="outp", bufs=3))

        ps_ctx = ExitStack()
        ps2 = ps_ctx.enter_context(tc.tile_pool(name="ps2", bufs=2, space="PSUM"))
        ps1_ctx = ExitStack()
        ps1 = ps1_ctx.enter_context(tc.tile_pool(name="ps1", bufs=2, space="PSUM"))

        def rope(dst, h, j, proj_ps):
            """dst slice = rope(proj_ps) using cos/sinm tables (chunk j)."""
            sl = slice(j * 512, (j + 1) * 512)
            qs = rope_pool.tile([128, 512], bf16, name=f"qs_{h}_{j}", tag="qs")
            nc.vector.tensor_copy(qs, proj_ps)  # PSUM f32 -> SBUF bf16
            rot = rope_pool.tile([128, 512], bf16, name=f"rot_{h}_{j}", tag="rot")
            # rotate_half as partition-shifted copies (sign folded into sinM)
            nc.vector.tensor_copy(rot[0:64, :], qs[64:128, :])
            nc.vector.tensor_copy(rot[64:128, :], qs[0:64, :])
            nc.vector.tensor_mul(qs, qs, cos_sb[:, sl])      # in-place
            nc.vector.tensor_mul(rot, rot, sinm_sb[:, sl])   # in-place
            nc.vector.tensor_add(dst, qs, rot)

        wo_pool = None
        wo_ec = {}
        bo_sb = ones_row = None

        for j in range(nj):
            # --- xT chunk j (+ round-0: interleave weight loads) ---
            xt_j = xt_pool.tile([128, NE, 512], bf16, name=f"xt_{j}", tag="xt")
            for e2 in range(NE // 2):
                load_xt(xt_j, j, e2)
                if j == 0:
                    # q/k weights first (their chains run first), wv behind;
                    # spread across both HWDGE queues and both copy engines
                    load_w("wq", wq_d, e2, nc.scalar, nc.vector)
                    load_w("wk", wk_d, e2, nc.sync,
                           nc.vector if e2 % 2 else nc.scalar)
            if j == 0:
                for e2 in range(NE // 2):
                    load_w("wv", wv_d, e2,
                           nc.scalar if e2 % 2 else nc.sync, nc.vector)
                load_tables()

            def qk_proj():
                qt_j = qtj_pool.tile([128, HPG, 512], bf16, name=f"qt_{j}",
                                     tag="qtj")
                for h in range(HPG):
                    for nm in ("wq", "wk"):
                        pp = ps1.tile([128, 512], f32, name=f"pp_{nm}_{h}_{j}",
                                      tag="proj")
                        for e in range(NE):
                            nc.tensor.matmul(
                                pp, w_sb[nm][:, e, h * 128:(h + 1) * 128],
                                xt_j[:, e, :], start=(e == 0), stop=(e == NE - 1))
                        if nm == "wq":
                            rope(qt_j[:, h, :], h, j, pp)
                        else:
                            rope(kt[:, h, j * 512:(j + 1) * 512], h, j, pp)
                return qt_j

            def v_proj():
                for sb in range(4):
                    i_blk = j * 4 + sb
                    vp = ps1.tile([128, DPG], f32, name=f"vp_{i_blk}", tag="proj")
                    for e in range(NE):
                        nc.tensor.matmul(
                            vp, xt_j[:, e, sb * 128:(sb + 1) * 128],
                            w_sb["wv"][:, e, :], start=(e == 0), stop=(e == NE - 1))
                    nc.vector.tensor_copy(v_sb[:, i_blk, :], vp)

            qt_j = qk_proj()
            v_proj()

            if j == nj - 1:
                # last round: projections done with xT/w/tables -> free the
                # right side and stage the out-projection weights; the DMAs
                # overlap this round's attention. The projection PSUM banks
                # are also done -> hand them to the out-projection pool so
                # out-proj rows that only need earlier q-chunks can overlap
                # this round's attention.
                ps1_ctx.close()
                ph2.close()
                wo_pool = ctx.enter_context(tc.tile_pool(name="wop", bufs=1))
                if has_bias:
                    ones_row = wo_pool.tile([1, 128], bf16, name="ones_row")
                    nc.vector.memset(ones_row, 1.0)
                    bo_f32 = wo_pool.tile([1, EMB], f32, name="bo_f32")
                    nc.scalar.dma_start(bo_f32, bo_d[:])
                    bo_sb = wo_pool.tile([1, EMB], bf16, name="bo_sb")
                    nc.gpsimd.tensor_copy(bo_sb, bo_f32)
                for ec in range(EMB // 512):
                    wo_ec[ec] = wo_pool.tile([128, HPG, 512], bf16, name=f"wo_{ec}")
                for ec2 in range(EMB // 1024):
                    for h in range(HPG):
                        st = stage_p.tile([128, 1024], f32, name=f"wost_{ec2}_{h}",
                                          tag="stg_x")
                        eng = nc.scalar if (ec2 + h) % 2 else nc.sync
                        eng.dma_start(
                            st, wo_d[h * 128:(h + 1) * 128,
                                     ec2 * 1024:(ec2 + 1) * 1024])
                        # gpsimd (idle) so these casts don't head-of-line
                        # block round-3 attention's DVE/ACT work
                        nc.gpsimd.tensor_copy(wo_ec[2 * ec2][:, h, :],
                                              st[:, 0:512])
                        nc.gpsimd.tensor_copy(wo_ec[2 * ec2 + 1][:, h, :],
                                              st[:, 512:1024])

            # --- attention for all heads, q-chunk j ---
            for h in range(HPG):
                nblk = 4 * j + 4
                ot = ps2.tile([128, 512], f32, name=f"ot_{h}_{j}", tag="ot")
                rs = ps2.tile([1, 512], f32, name=f"rs_{h}_{j}", tag="rs", bufs=1)
                for i in range(nblk):
                    m = i - 4 * j  # diagonal index (>=0 on the 4 trailing blocks)
                    qoff = max(m, 0) * 128
                    n = 512 - qoff
                    qs_l = slice(j * 512 + qoff, (j + 1) * 512)
                    st_ps = ps2.tile([128, 512], f32, name=f"st_{h}_{j}_{i}", tag="st", bufs=3)
                    nc.tensor.matmul(
                        st_ps[:, 0:n], kt[:, h, i * 128:(i + 1) * 128],
                        qt_j[:, h, qoff:512], start=True, stop=True)
                    pt = pt_pool.tile([128, 512], bf16, name=f"pt_{h}_{j}_{i}",
                                      tag="pt")
                    nc.scalar.activation(pt[:, 0:n], st_ps[:, 0:n], EXP, scale=SCALE)
                    if m >= 0:  # triangle mask on the leading 128 q columns
                        nc.vector.tensor_mul(pt[:, 0:128], pt[:, 0:128], mask_sb)
                    nc.tensor.matmul(ot[:, qoff:512],
                                     v_sb[:, i, h * 128:(h + 1) * 128], pt[:, 0:n],
                                     start=(i == 0), stop=(i == nblk - 1))
                    nc.tensor.matmul(rs[:, qoff:512], ones_col, pt[:, 0:n],
                                     start=(i == 0), stop=(i == nblk - 1))
                # normalize: yt = ot / rowsum (broadcast along partitions)
                rsf = sm_pool.tile([1, 512], f32, name=f"rsf_{h}_{j}", tag="rsf")
                nc.vector.tensor_copy(rsf, rs)
                nc.vector.reciprocal_approx_fast(rsf, rsf)
                rb = sm_pool.tile([128, 512], f32, name=f"rb_{h}_{j}", tag="rb")
                nc.gpsimd.partition_broadcast(rb, rsf)
                nc.vector.tensor_mul(yt[:, h, j * 512:(j + 1) * 512], ot, rb)

        # ---- output projection (ps2 still open: ps3 gets the 2 freed banks,
        # letting early row-blocks overlap the tail of round-3 attention) ----
        with tc.tile_pool(name="ps3", bufs=2, space="PSUM") as ps3:
            for sb in range(nsb):
                ssl = slice(sb * 128, (sb + 1) * 128)
                for ec2 in range(EMB // 1024):
                    ob = out_pool.tile([128, 1024], f32, name=f"ob_{sb}_{ec2}",
                                       tag="ob")
                    for half in range(2):
                        ec = 2 * ec2 + half
                        esl = slice(ec * 512, (ec + 1) * 512)
                        op = ps3.tile([128, 512], f32, name=f"op_{sb}_{ec}",
                                      tag="op")
                        for h in range(HPG):
                            nc.tensor.matmul(op, yt[:, h, ssl], wo_ec[ec][:, h, :],
                                             start=(h == 0),
                                             stop=(not has_bias and h == HPG - 1))
                        if has_bias:
                            nc.tensor.matmul(op, ones_row, bo_sb[:, esl],
                                             start=False, stop=True)
                        nc.scalar.copy(ob[:, half * 512:(half + 1) * 512], op)
                    eng = nc.sync if (sb + ec2) % 2 == 0 else nc.scalar
                    eng.dma_start(
                        out_d[ssl, ec2 * 1024:(ec2 + 1) * 1024], ob)
        ps_ctx.close()


_NC_CACHE = {}


def _get_nc(seq=SEQ, has_bias=False):
    key = (seq, has_bias)
    if key not in _NC_CACHE:
        _NC_CACHE[key] = build(seq, has_bias)
    return _NC_CACHE[key]


def make_in_maps(x, w_kv, w_q, w_o, b_o, seq=SEQ):
    """Shard full inputs into the 8 per-core input dicts."""
    cos_t, sinm, mask_t = _host_tables(seq)
    cos_t = np.ascontiguousarray(cos_t)
    sinm = np.ascontiguousarray(sinm)
    mask_t = np.ascontiguousarray(mask_t)
    zeros_bo = np.zeros((1, EMB), np.float32)
    xts = [np.ascontiguousarray(np.asarray(x[b], np.float32).T) for b in range(BATCH)]
    in_maps = []
    for c in range(N_CORES):
        b, g = divmod(c, GROUPS)
        d0 = g * DPG
        in_maps.append({
            "xt": xts[b],
            "wq": np.ascontiguousarray(w_q[:, d0:d0 + DPG], np.float32),
            "wk": np.ascontiguousarray(w_kv[:, d0:d0 + DPG], np.float32),
            "wv": np.ascontiguousarray(w_kv[:, EMB + d0:EMB + d0 + DPG], np.float32),
            "wo": np.ascontiguousarray(w_o[d0:d0 + DPG, :], np.float32),
            "bo": (np.ascontiguousarray(b_o.reshape(1, EMB), np.float32)
                   if g == 0 else zeros_bo),
            "cosT": cos_t,
            "sinM": sinm,
            "maskT": mask_t,
        })
    return in_maps


def kernel(x, w_kv, w_q, w_o, b_o):
    from concourse.bass_utils import run_bass_kernel_spmd

    x = np.asarray(x, np.float32)
    nc = _get_nc(SEQ, has_bias=bool(np.any(np.asarray(b_o))))
    in_maps = make_in_maps(x, np.asarray(w_kv, np.float32),
                           np.asarray(w_q, np.float32),
                           np.asarray(w_o, np.float32),
                           np.asarray(b_o, np.float32), SEQ)
    res = run_bass_kernel_spmd(nc, in_maps, core_ids=list(range(N_CORES)))
    parts = [res.results[c]["out"] for c in range(N_CORES)]
    out = np.stack(
        [parts[0] + parts[1] + parts[2] + parts[3],
         parts[4] + parts[5] + parts[6] + parts[7]], axis=0)
    return out.astype(np.float32)



# revision 8
# speedup vs baseline: 1.2142x; 1.2142x over previous
"""Multi-head attention (QKV proj + RoPE + causal SDPA + out proj) on 8 TRN2 cores.

Sharding: core c = 4*b + g handles batch b (of 2) and head-group g (of 4, i.e.
4 heads = 512 feature dims). Per-core device kernel computes, for its batch:
    q/k/v projections for its 4 heads (column-sharded w_q / w_kv),
    RoPE on q and k, causal softmax attention,
    partial output projection with its 512 rows of w_o (+ bias on g==0 cores).
Host gathers by summing the 4 per-group partials per batch (the "all-reduce"
for the row-sharded w_o, done as the unshard step).

All inputs are cast to bf16 on the host so DMAs land directly in their final
SBUF tiles (no staging copies / on-device casts). Output is bf16, upcast and
summed on the host.

Device dataflow is feature-major: the host passes x pre-transposed (xT[e, s]);
projections produce qT/kT in [head_dim, seq] layout and v in [seq, head_dim]
layout, which is exactly what the S^T = K.Q^T and O^T = V^T.P^T matmuls need.

Engine balance: PE does only matmuls; ACT does the softmax exp (and the
PSUM->SBUF copies feeding RoPE / v, emitted during projection rounds where it
is otherwise idle); DVE does RoPE arithmetic, the P-accumulation that replaces
per-block rowsum matmuls, and PSUM evacuation; GpSimd broadcasts 1/rowsum.
Because exp throughput (1.2 G elem/s/lane) is slightly below the 2-matmul PE
pace, independent matmul chains (next chunk's q/k projections, then the output
projection) are interleaved into the attention instruction stream so the PE
never waits on ACT.
"""

import sys

import numpy as np

sys.path.insert(0, "/opt/trn_rl_repo")

EMB = 2048
SEQ = 2048
N_HEAD = 16
HD = 128
BATCH = 2
N_CORES = 8
GROUPS = 4  # head groups (tensor-parallel dimension)
HPG = N_HEAD // GROUPS  # heads per group = 4
DPG = HPG * HD  # feature dims per group = 512
NE = EMB // 128  # 16 e-blocks
SCALE = float(HD) ** -0.5


def _host_tables(seq):
    """cos / sign-folded sin RoPE tables in [d, s] layout + triangle mask."""
    d = HD
    inv = 1.0 / (10000.0 ** (np.arange(0, d, 2, dtype=np.float64) / d))  # [64]
    pos = np.arange(seq, dtype=np.float64)[None, :] * inv[:, None]  # [64, s]
    ang = np.concatenate([pos, pos], axis=0)  # [128, s]
    cos_t = np.cos(ang)
    sin = np.sin(ang)
    # rot is built as a plain partition swap (rot[0:64]=q[64:128], rot[64:128]=q[0:64]);
    # the rotate_half sign lives in the sin table instead.
    sinm = np.concatenate([-sin[:64], sin[64:]], axis=0)
    # triangle mask for the diagonal 128x128 block: keep (ko, qo) iff qo >= ko
    ko = np.arange(128)[:, None]
    qo = np.arange(128)[None, :]
    mask_t = (qo >= ko).astype(np.float64)
    return cos_t, sinm, mask_t


def build(seq=SEQ, has_bias=False):
    """Build the per-core Bass program. Returns the compiled Bacc module."""
    import concourse.bacc as bacc
    import concourse.tile as tile
    from concourse import mybir

    bf16 = mybir.dt.bfloat16

    assert seq % 512 == 0

    nc = bacc.Bacc("TRN2", target_bir_lowering=False, debug=False,
                   num_devices=N_CORES, name="mha8")

    xt_d = nc.dram_tensor("xt", [EMB, seq], bf16, kind="ExternalInput")
    wq_d = nc.dram_tensor("wq", [EMB, DPG], bf16, kind="ExternalInput")
    wk_d = nc.dram_tensor("wk", [EMB, DPG], bf16, kind="ExternalInput")
    wv_d = nc.dram_tensor("wv", [EMB, DPG], bf16, kind="ExternalInput")
    wo_d = nc.dram_tensor("wo", [DPG, EMB], bf16, kind="ExternalInput")
    bo_d = nc.dram_tensor("bo", [1, EMB], bf16, kind="ExternalInput")
    cos_d = nc.dram_tensor("cosT", [HD, seq], bf16, kind="ExternalInput")
    sinm_d = nc.dram_tensor("sinM", [HD, seq], bf16, kind="ExternalInput")
    mask_d = nc.dram_tensor("maskT", [128, 128], bf16, kind="ExternalInput")
    out_d = nc.dram_tensor("out", [seq, EMB], bf16, kind="ExternalOutput")

    with tile.TileContext(nc) as tc:
        _emit(nc, tc, tile, mybir, seq,
              xt_d, wq_d, wk_d, wv_d, wo_d, bo_d, cos_d, sinm_d, mask_d, out_d,
              has_bias)
    nc.compile()
    return nc


def _emit(nc, tc, tile, mybir, seq,
          xt_d, wq_d, wk_d, wv_d, wo_d, bo_d, cos_d, sinm_d, mask_d, out_d,
          has_bias):
    from contextlib import ExitStack

    f32 = mybir.dt.float32
    bf16 = mybir.dt.bfloat16
    EXP = mybir.ActivationFunctionType.Exp
    COPY = mybir.ActivationFunctionType.Copy
    nj = seq // 512  # 512-wide q chunks
    nsb = seq // 128

    ctx = ExitStack()
    with ctx:
        persist = ctx.enter_context(tc.tile_pool(name="persist", bufs=1))
        wpool = ctx.enter_context(tc.tile_pool(name="wpool", bufs=1, side="right"))

        # ---- constants / persistent tiles ----
        ones_col = persist.tile([128, 1], bf16, name="ones_col")
        nc.vector.memset(ones_col, 1.0)
        dummy = persist.tile([1, 1], f32, name="dummy")
        nc.vector.memset(dummy, 0.0)
        # pre-trigger the exp ACT table load so it overlaps the input DMAs
        nc.scalar.activation(dummy, dummy, EXP)

        mask_sb = persist.tile([128, 128], bf16, name="mask_sb")
        cos_sb = wpool.tile([128, seq], bf16, name="cos_sb")
        sinm_sb = wpool.tile([128, seq], bf16, name="sinm_sb")

        kt = persist.tile([128, HPG, seq], bf16, name="kt")     # [d, h, s]
        yt = persist.tile([128, HPG, seq], bf16, name="yt")
        v_sb = persist.tile([128, nsb, DPG], bf16, name="v_sb")  # [s_in, blk, d]

        w_sb = {nm: wpool.tile([128, NE, DPG], bf16, name=f"{nm}_sb")
                for nm in ("wq", "wk", "wv")}
        wo_sb = wpool.tile([128, HPG, EMB], bf16, name="wo_sb")

        xt_pool = ctx.enter_context(tc.tile_pool(name="xt", bufs=2, side="right"))
        qtj_pool = ctx.enter_context(tc.tile_pool(name="qtj", bufs=2))
        rope_pool = ctx.enter_context(tc.tile_pool(name="rope", bufs=2))
        pt_pool = ctx.enter_context(tc.tile_pool(name="pt", bufs=4))
        acc_pool = ctx.enter_context(tc.tile_pool(name="accp", bufs=2))
        sm_pool = ctx.enter_context(tc.tile_pool(name="sm", bufs=2))
        ob_pool = ctx.enter_context(tc.tile_pool(name="obp", bufs=2))

        st_pool = ctx.enter_context(tc.tile_pool(name="stp", bufs=3, space="PSUM"))
        ot_pool = ctx.enter_context(tc.tile_pool(name="otp", bufs=2, space="PSUM"))
        rs_pool = ctx.enter_context(tc.tile_pool(name="rsp", bufs=1, space="PSUM"))
        ps1_ctx = ExitStack()
        ps1 = ps1_ctx.enter_context(tc.tile_pool(name="ps1", bufs=2, space="PSUM"))

        # ---- DMA helpers (all direct bf16, no staging) ----
        def load_w(nm, wd, eng):
            # [2048, 512] -> [128, 16, 512]; two half DMAs
            for half in range(2):
                src = wd[half * 1024:(half + 1) * 1024, :].rearrange(
                    "(e p) d -> p e d", p=128)
                eng.dma_start(w_sb[nm][:, half * 8:(half + 1) * 8, :], src)

        def load_xt(j, eng):
            xt_j = xt_pool.tile([128, NE, 512], bf16, name=f"xt_{j}", tag="xt")
            for half in range(2):
                src = xt_d[half * 1024:(half + 1) * 1024,
                           j * 512:(j + 1) * 512].rearrange(
                    "(e p) s -> p e s", p=128)
                eng.dma_start(xt_j[:, half * 8:(half + 1) * 8, :], src)
            return xt_j

        def load_wo():
            for half in range(2):
                src = wo_d[half * 256:(half + 1) * 256, :].rearrange(
                    "(h p) e -> p h e", p=128)
                eng = nc.scalar if half else nc.sync
                eng.dma_start(wo_sb[:, half * 2:(half + 1) * 2, :], src)

        # ---- compute helpers ----
        def rope(dst, h, j, pp, tag):
            """dst = rope(pp) (chunk j); pp is the f32 PSUM projection."""
            sl = slice(j * 512, (j + 1) * 512)
            # ACT evacuates PSUM (idle during projection work); DVE does the rest
            qs = rope_pool.tile([128, 512], bf16, name=f"qs_{tag}", tag="qs")
            nc.scalar.activation(qs, pp, COPY)
            rot = rope_pool.tile([128, 512], bf16, name=f"rot_{tag}", tag="rot")
            # rotate_half as partition-shifted copies (sign folded into sinM);
            # TensorTensor ops must be partition-aligned, plain copies may shift
            nc.vector.tensor_copy(rot[0:64, :], qs[64:128, :])
            nc.vector.tensor_copy(rot[64:128, :], qs[0:64, :])
            nc.vector.tensor_mul(rot, rot, sinm_sb[:, sl])   # in-place
            nc.vector.tensor_mul(qs, qs, cos_sb[:, sl])      # in-place
            nc.vector.tensor_add(dst, qs, rot)

        qt_tiles = {}

        def qk_chain(j, h, nm, xt_j):
            """One 16-matmul projection chain + rope for (chunk j, head h)."""
            if j not in qt_tiles:
                qt_tiles[j] = qtj_pool.tile([128, HPG, 512], bf16,
                                            name=f"qt_{j}", tag="qtj")
            pp = ps1.tile([128, 512], f32, name=f"pp_{nm}_{h}_{j}", tag="proj")
            for e in range(NE):
                nc.tensor.matmul(pp, w_sb[nm][:, e, h * 128:(h + 1) * 128],
                                 xt_j[:, e, :], start=(e == 0), stop=(e == NE - 1))
            if nm == "wq":
                rope(qt_tiles[j][:, h, :], h, j, pp, f"q{h}_{j}")
            else:
                rope(kt[:, h, j * 512:(j + 1) * 512], h, j, pp, f"k{h}_{j}")

        def v_chain(j, sb, xt_j):
            i_blk = j * 4 + sb
            vp = ps1.tile([128, DPG], f32, name=f"vp_{i_blk}", tag="proj")
            for e in range(NE):
                nc.tensor.matmul(vp, xt_j[:, e, sb * 128:(sb + 1) * 128],
                                 w_sb["wv"][:, e, :], start=(e == 0),
                                 stop=(e == NE - 1))
            nc.scalar.activation(v_sb[:, i_blk, :], vp, COPY)

        bo_sb = ones_row = None

        def op_block(sb, pool, out_eng):
            """Output projection for seq row-block sb: [128, 2048] partial."""
            ssl = slice(sb * 128, (sb + 1) * 128)
            ob = ob_pool.tile([128, EMB], bf16, name=f"ob_{sb}", tag="ob")
            for ec in range(EMB // 512):
                esl = slice(ec * 512, (ec + 1) * 512)
                op = pool.tile([128, 512], f32, name=f"op_{sb}_{ec}", tag="proj")
                for h in range(HPG):
                    nc.tensor.matmul(op, yt[:, h, ssl], wo_sb[:, h, esl],
                                     start=(h == 0),
                                     stop=(not has_bias and h == HPG - 1))
                if has_bias:
                    nc.tensor.matmul(op, ones_row, bo_sb[:, esl],
                                     start=False, stop=True)
                nc.vector.tensor_copy(ob[:, esl], op)
            # during attention the scalar/ACT ring must stay free for exps (a
            # waiting DMA at the ACT queue head would block them) -> sync only
            out_eng.dma_start(out_d[ssl, :], ob)

        # ---- preamble DMAs ----
        # scalar ring: wq, wk first (first chains), then wv, mask; sync ring:
        # xt0, rope tables (needed ~4us in).
        load_w("wq", wq_d, nc.scalar)
        xt_cur = load_xt(0, nc.sync)
        load_w("wk", wk_d, nc.scalar)
        nc.sync.dma_start(cos_sb, cos_d[:])
        nc.sync.dma_start(sinm_sb, sinm_d[:])
        load_w("wv", wv_d, nc.scalar)
        nc.scalar.dma_start(mask_sb, mask_d[:])
        if has_bias:
            ones_row = persist.tile([1, 128], bf16, name="ones_row")
            nc.vector.memset(ones_row, 1.0)
            bo_sb = persist.tile([1, EMB], bf16, name="bo_sb")
            nc.scalar.dma_start(bo_sb, bo_d[:])

        # round 0 projections run before any attention exists to interleave
        for h in range(HPG):
            for nm in ("wq", "wk"):
                qk_chain(0, h, nm, xt_cur)

        xt_next = load_xt(1, nc.sync)
        load_wo()

        # ---- main rounds ----
        for j in range(nj):
            for sb in range(4):
                v_chain(j, sb, xt_cur)

            if j == nj - 1:
                # projections done: free ps1's banks and reopen them for the
                # interleaved output projection
                ps1_ctx.close()
                ps3 = ctx.enter_context(tc.tile_pool(name="ps3", bufs=2,
                                                     space="PSUM"))

            # filler queue: independent PE chains interleaved into attention
            # so the PE keeps running while ACT works through the exps
            fillers = []
            if j + 1 < nj:
                xt_for_next = xt_next
                for h in range(HPG):
                    for nm in ("wq", "wk"):
                        fillers.append((lambda jj=j + 1, hh=h, nn=nm,
                                        xx=xt_for_next:
                                        qk_chain(jj, hh, nn, xx)))
            else:
                for sb in range(12):
                    fillers.append(lambda s=sb: op_block(s, ps3, nc.sync))
            fill_idx = 0
            # PE deficit per attention block ~290ns; one filler chain ~3.4us
            blocks_per_fill = max(1, (len(fillers) and
                                      (4 * (4 * j + 4)) // len(fillers)))

            blk_count = 0
            for h in range(HPG):
                nblk = 4 * j + 4
                ot = ot_pool.tile([128, 512], f32, name=f"ot_{h}_{j}", tag="ot")
                acc = acc_pool.tile([128, 512], bf16, name=f"acc_{h}_{j}",
                                    tag="acc")
                for i in range(nblk):
                    m = i - 4 * j  # diagonal index (>=0 on the 4 trailing blocks)
                    qoff = max(m, 0) * 128
                    n = 512 - qoff
                    st = st_pool.tile([128, 512], f32, name=f"st_{h}_{j}_{i}",
                                      tag="st", bufs=3)
                    nc.tensor.matmul(
                        st[:, 0:n], kt[:, h, i * 128:(i + 1) * 128],
                        qt_tiles[j][:, h, qoff:512], start=True, stop=True)
                    pt = pt_pool.tile([128, 512], bf16, name=f"pt_{h}_{j}_{i}",
                                      tag="pt")
                    nc.scalar.activation(pt[:, 0:n], st[:, 0:n], EXP, scale=SCALE)
                    # filler sits between this block's S and PV: the PE chews
                    # through it while ACT finishes the exp, so PV never waits
                    blk_count += 1
                    if (fill_idx < len(fillers)
                            and blk_count % blocks_per_fill == 0):
                        fillers[fill_idx]()
                        fill_idx += 1
                    if m >= 0:  # triangle mask on the leading 128 valid q cols
                        nc.vector.tensor_mul(pt[:, 0:128], pt[:, 0:128], mask_sb)
                    if i == 0:
                        nc.vector.tensor_copy(acc, pt)
                    else:
                        nc.vector.tensor_add(acc[:, qoff:512], acc[:, qoff:512],
                                             pt[:, 0:n])
                    nc.tensor.matmul(ot[:, qoff:512],
                                     v_sb[:, i, h * 128:(h + 1) * 128],
                                     pt[:, 0:n],
                                     start=(i == 0), stop=(i == nblk - 1))
                # single rowsum matmul over the accumulated P
                rs = rs_pool.tile([1, 512], f32, name=f"rs_{h}_{j}", tag="rs")
                nc.tensor.matmul(rs, ones_col, acc, start=True, stop=True)
                rsf = sm_pool.tile([1, 512], f32, name=f"rsf_{h}_{j}", tag="rsf")
                nc.vector.tensor_copy(rsf, rs)
                nc.vector.reciprocal_approx_fast(rsf, rsf)
                rb = sm_pool.tile([128, 512], f32, name=f"rb_{h}_{j}", tag="rb")
                nc.gpsimd.partition_broadcast(rb, rsf)
                nc.vector.tensor_mul(yt[:, h, j * 512:(j + 1) * 512], ot, rb)
            while fill_idx < len(fillers):
                fillers[fill_idx]()
                fill_idx += 1

            xt_cur = xt_next
            if j + 2 < nj:
                xt_next = load_xt(j + 2, nc.sync)

        # remaining output-projection row blocks (need round-3 attention);
        # attention is over so the scalar ring is free again
        for sb in range(12, nsb):
            op_block(sb, ps3, nc.scalar if sb % 2 else nc.sync)


_NC_CACHE = {}


def _get_nc(seq=SEQ, has_bias=False):
    key = (seq, has_bias)
    if key not in _NC_CACHE:
        _NC_CACHE[key] = build(seq, has_bias)
    return _NC_CACHE[key]


def make_in_maps(x, w_kv, w_q, w_o, b_o, seq=SEQ):
    """Shard full inputs into the 8 per-core input dicts (all bf16)."""
    import ml_dtypes

    bf = ml_dtypes.bfloat16
    cos_t, sinm, mask_t = _host_tables(seq)
    cos_t = np.ascontiguousarray(cos_t.astype(bf))
    sinm = np.ascontiguousarray(sinm.astype(bf))
    mask_t = np.ascontiguousarray(mask_t.astype(bf))
    zeros_bo = np.zeros((1, EMB), bf)
    x = np.asarray(x, np.float32)
    w_kv = np.asarray(w_kv, np.float32)
    w_q = np.asarray(w_q, np.float32)
    w_o = np.asarray(w_o, np.float32)
    b_o = np.asarray(b_o, np.float32)
    xts = [np.ascontiguousarray(x[b].T.astype(bf)) for b in range(BATCH)]
    in_maps = []
    for c in range(N_CORES):
        b, g = divmod(c, GROUPS)
        d0 = g * DPG
        in_maps.append({
            "xt": xts[b],
            "wq": np.ascontiguousarray(w_q[:, d0:d0 + DPG].astype(bf)),
            "wk": np.ascontiguousarray(w_kv[:, d0:d0 + DPG].astype(bf)),
            "wv": np.ascontiguousarray(
                w_kv[:, EMB + d0:EMB + d0 + DPG].astype(bf)),
            "wo": np.ascontiguousarray(w_o[d0:d0 + DPG, :].astype(bf)),
            "bo": (np.ascontiguousarray(b_o.reshape(1, EMB).astype(bf))
                   if g == 0 else zeros_bo),
            "cosT": cos_t,
            "sinM": sinm,
            "maskT": mask_t,
        })
    return in_maps


def gather_out(results):
    """Sum the 4 per-group bf16 partials per batch into the full f32 output."""
    parts = [np.asarray(results[c]["out"], np.float32) for c in range(N_CORES)]
    return np.stack([parts[0] + parts[1] + parts[2] + parts[3],
                     parts[4] + parts[5] + parts[6] + parts[7]], axis=0)


def kernel(x, w_kv, w_q, w_o, b_o):
    from concourse.bass_utils import run_bass_kernel_spmd

    nc = _get_nc(SEQ, has_bias=bool(np.any(np.asarray(b_o))))
    in_maps = make_in_maps(x, w_kv, w_q, w_o, b_o, SEQ)
    res = run_bass_kernel_spmd(nc, in_maps, core_ids=list(range(N_CORES)))
    return gather_out(res.results).astype(np.float32)
